# revision 7
# baseline (speedup 1.0000x reference)
"""Trainium2 Bass kernel for GroupedQueryAttention (cost-model-optimized v4).

Sharding: 8 cores; core c owns KV head g=c and Q heads 4c..4c+3, both batch
elements. Each core computes its [2, 2048, 256] output slice; host concats.

Host prep (pure layout, no FLOPs beyond the 1/sqrt(HD) weight fold):
  - hsT packed as [B, NSC, 128, NDT*512]: chunk (b, sc) is ONE [128, 8192]
    DMA whose col-block dt holds hsT[dt*128:.., sc*512:..] — 4 DMAs/batch.
  - W packed as [NDT/4, 128, 4*384]: 4 DMAs, col-block k = d-tile 4i+k,
    each [wq(256) | wkv(128)] fused.
DMA count is what matters: every non-Pool DMA serializes ~0.6us on the
shared HWDGE device.

All matmul operands fp16 (PSUM accumulates fp32). Per batch:
  A) qT (2x[128,2048]) and kvT [128,2048] = W^T @ hsT, 16 d-tiles PSUM-
     accumulated; DVE bias-add casts to fp16. kv chains first, then qc0;
     qc1 chains deferred (heads 2-3 run ~80us later). kth = K copied to
     partitions 64:127 for odd heads.
  B) V^T PE-transposed to natural [V|1] slices (ones col -> denominator).
  C) Per (head, q-half): scores S^T per kt (2x512-wide matmuls, PSUM bank
     limit), ACT exp -> fp16 SBUF (the global bottleneck: 256 x ~1us).
     PV natural: ctx[q,65] += expS^T_slice^T @ [V|1] over kt (stationary =
     expS^T, moving = 65 cols; output q-natural, no back-transpose).
     DVE: reciprocal of col 64, scale into out tile.
"""

import sys
from contextlib import ExitStack

import numpy as np

sys.path.insert(0, "/opt/trn_rl_repo")

import concourse.bass as bass  # noqa: E402
import concourse.bacc as bacc  # noqa: E402
import concourse.tile as tile  # noqa: E402
from concourse import mybir  # noqa: E402
from concourse.bass_utils import run_bass_kernel_spmd  # noqa: E402

B = 2
S = 2048
D = 2048
HD = 64
NCORES = 8
QH = 4           # q heads per core
MCOLS = QH * HD  # 256 output cols per core
WCOLS = MCOLS + 128  # fused [wq|wkv] cols

F16 = mybir.dt.float16
F32 = mybir.dt.float32
Exp = mybir.ActivationFunctionType.Exp

NDT = 16         # d tiles of 128
NSC = 4          # s chunks of 512 per batch (projection)
NKT = 16         # s_k tiles of 128


def build_nc():
    nc = bacc.Bacc("TRN2", target_bir_lowering=False, debug=False)

    hst_d = nc.dram_tensor("hst", [B, NSC, 128, NDT * 512], F16,
                           kind="ExternalInput")
    w_d = nc.dram_tensor("w", [NDT // 4, 128, 4 * WCOLS], F16,
                         kind="ExternalInput")
    bq_d = nc.dram_tensor("bq", [128, 2], F32, kind="ExternalInput")
    bkv_d = nc.dram_tensor("bkv", [128, 1], F32, kind="ExternalInput")
    id_d = nc.dram_tensor("ident", [128, 128], F16, kind="ExternalInput")
    out_d = nc.dram_tensor("out", [B, S, MCOLS], F16, kind="ExternalOutput")

    with tile.TileContext(nc) as tc, ExitStack() as ctx:
        const = ctx.enter_context(tc.tile_pool(name="const", bufs=1))
        wqp = ctx.enter_context(tc.tile_pool(name="wqp", bufs=4))
        hstp = ctx.enter_context(tc.tile_pool(name="hstp", bufs=4))
        qtp = ctx.enter_context(tc.tile_pool(name="qtp", bufs=4))
        kvtp = ctx.enter_context(tc.tile_pool(name="kvtp", bufs=2))
        kthp = ctx.enter_context(tc.tile_pool(name="kthp", bufs=2))
        v1p = ctx.enter_context(tc.tile_pool(name="v1p", bufs=2))
        expp = ctx.enter_context(tc.tile_pool(name="expp", bufs=28))
        recp = ctx.enter_context(tc.tile_pool(name="recp", bufs=4))
        outp = ctx.enter_context(tc.tile_pool(name="outp", bufs=32))
        psA = ctx.enter_context(tc.tile_pool(name="psA", bufs=2, space="PSUM"))
        psS = ctx.enter_context(tc.tile_pool(name="psS", bufs=2, space="PSUM"))
        psC = ctx.enter_context(tc.tile_pool(name="psC", bufs=2, space="PSUM"))

        # consts via Pool (SWDGE, off the shared HWDGE device)
        ident = const.tile([128, 128], F16, tag="ident")
        nc.gpsimd.dma_start(out=ident[:], in_=id_d[:])
        bq_sb = const.tile([128, 2], F32, tag="bq")
        nc.gpsimd.dma_start(out=bq_sb[:], in_=bq_d[:])
        bkv_sb = const.tile([128, 1], F32, tag="bkv")
        nc.gpsimd.dma_start(out=bkv_sb[:], in_=bkv_d[:])
        zb = const.tile([128, 1], F32, tag="zb")
        nc.vector.memset(zb[:], 0.0)

        # packed weights via SP: 4 DMAs; col-block k = d-tile 4i+k
        wt = []
        for i in range(NDT // 4):
            w = wqp.tile([128, 4 * WCOLS], F16, tag="wq")
            nc.sync.dma_start(out=w[:], in_=w_d[i])
            wt.append(w)
        wq_sb = [wt[dt_ // 4][:, (dt_ % 4) * WCOLS:(dt_ % 4) * WCOLS + MCOLS]
                 for dt_ in range(NDT)]
        wkv_sb = [wt[dt_ // 4][:, (dt_ % 4) * WCOLS + MCOLS:
                               (dt_ % 4) * WCOLS + WCOLS]
                  for dt_ in range(NDT)]

        for b in range(B):
            # ---- Phase A: projections from host-packed hsT chunks ----
            qT = [qtp.tile([128, S], F16, tag="qt", name=f"qT{b}_{i}")
                  for i in range(2)]
            kvT = kvtp.tile([128, S], F16, tag="kvt")
            hs_ch = []
            for sc in range(NSC):
                t = hstp.tile([128, NDT * 512], F16, tag="hst",
                              name=f"hsT{b}_{sc}")
                nc.sync.dma_start(out=t[:], in_=hst_d[b, sc])
                hs_ch.append(t)

            def chain(sc, wslices, cols, dst, bias):
                c0, c1 = sc * 512, (sc + 1) * 512
                ps = psA.tile([128, 512], F32, tag="pj")
                for dt_ in range(NDT):
                    nc.tensor.matmul(
                        ps[:], wslices[dt_] if cols is None
                        else wslices[dt_][:, cols[0]:cols[1]],
                        hs_ch[sc][:, dt_ * 512:(dt_ + 1) * 512],
                        start=(dt_ == 0), stop=(dt_ == NDT - 1),
                    )
                nc.vector.tensor_scalar_add(dst[:, c0:c1], ps[:], bias)

            # kv + qc0 first: they gate heads 0/1; qc1 deferred (heads 2/3)
            for sc in range(NSC):
                chain(sc, wkv_sb, None, kvT, bkv_sb[:])
                chain(sc, wq_sb, (0, 128), qT[0], bq_sb[:, 0:1])

            kth = kthp.tile([128, S], F16, tag="kth")
            nc.sync.dma_start(out=kth[64:128, :], in_=kvT[0:64, :])

            # ---- Phase B: natural V with ones column: [V|1] slices ----
            v1 = v1p.tile([128, NKT * 65], F16, tag="v1")
            for kt in range(NKT):
                pst = psA.tile([128, 64], F16, tag="pj")
                nc.tensor.transpose(
                    pst[:], kvT[64:128, kt * 128:(kt + 1) * 128],
                    ident[64:128, 64:128],
                )
                nc.vector.tensor_copy(v1[:, kt * 65:kt * 65 + 64], pst[:])
                nc.vector.memset(v1[:, kt * 65 + 64:kt * 65 + 65], 1.0)

            for sc in range(NSC):
                chain(sc, wq_sb, (128, 256), qT[1], bq_sb[:, 1:2])

            # ---- Phase C: attention ----
            outt = [outp.tile([128, MCOLS], F16, tag="out", name=f"outt{b}_{i}")
                    for i in range(16)]
            for h in range(QH):
                qrow = (h % 2) * 64
                qt = qT[h // 2]
                kmat = kvT if qrow == 0 else kth
                for half in range(2):
                    q0 = half * 1024
                    ex = []
                    for kt in range(NKT):
                        pss = psS.tile([128, 1024], F32, tag="sc")
                        for qc in range(2):
                            nc.tensor.matmul(
                                pss[:, qc * 512:(qc + 1) * 512],
                                kmat[qrow:qrow + 64, kt * 128:(kt + 1) * 128],
                                qt[qrow:qrow + 64,
                                   q0 + qc * 512:q0 + (qc + 1) * 512],
                                start=True, stop=True,
                            )
                        e = expp.tile([128, 1024], F16, tag="exp",
                                      name=f"ex{b}_{h}_{half}_{kt}")
                        nc.scalar.activation(e[:], pss[:], Exp, bias=zb[:])
                        ex.append(e)
                    for q8 in range(8):
                        cx = psC.tile([128, 65], F32, tag="cx")
                        for kt in range(NKT):
                            nc.tensor.matmul(
                                cx[:], ex[kt][:, q8 * 128:(q8 + 1) * 128],
                                v1[:, kt * 65:(kt + 1) * 65],
                                start=(kt == 0), stop=(kt == NKT - 1),
                            )
                        rec = recp.tile([128, 1], F32, tag="rec")
                        nc.vector.reciprocal(rec[:], cx[:, 64:65])
                        st_i = half * 8 + q8
                        nc.vector.tensor_scalar_mul(
                            outt[st_i][:, h * 64:(h + 1) * 64],
                            cx[:, 0:64], rec[:],
                        )
            for st_i in range(16):
                nc.sync.dma_start(
                    out=out_d[b, st_i * 128:(st_i + 1) * 128, :],
                    in_=outt[st_i][:],
                )

    nc.compile()
    return nc


def make_in_maps(hidden_states, Wq, bq, Wk, bk, Wv, bv):
    hs = np.asarray(hidden_states, dtype=np.float32)
    hst = hs.transpose(0, 2, 1).astype(np.float16)  # [B, D, S]
    # pack: hst_p[b, sc, p, dt*512+j] = hst[b, dt*128+p, sc*512+j]
    t = hst.reshape(B, NDT, 128, NSC, 512)
    hst_p = np.ascontiguousarray(t.transpose(0, 3, 2, 1, 4)
                                 .reshape(B, NSC, 128, NDT * 512))
    Wq = np.asarray(Wq, dtype=np.float32)
    bq = np.asarray(bq, dtype=np.float32)
    Wk = np.asarray(Wk, dtype=np.float32)
    bk = np.asarray(bk, dtype=np.float32)
    Wv = np.asarray(Wv, dtype=np.float32)
    bv = np.asarray(bv, dtype=np.float32)
    sc = 1.0 / np.sqrt(np.float32(HD))
    ident = np.eye(128, dtype=np.float16)
    in_maps = []
    for c in range(NCORES):
        qs = slice(c * MCOLS, (c + 1) * MCOLS)
        ks = slice(c * HD, (c + 1) * HD)
        bq_c = (bq[qs] * sc).reshape(2, 128).T
        wfull = np.concatenate(
            [Wq[:, qs] * sc, Wk[:, ks], Wv[:, ks]], axis=1
        ).astype(np.float16)                      # [D, WCOLS]
        # pack: w_p[i, p, k*WCOLS+j] = wfull[(4i+k)*128+p, j]
        wt = wfull.reshape(NDT // 4, 4, 128, WCOLS)
        w_p = np.ascontiguousarray(wt.transpose(0, 2, 1, 3)
                                   .reshape(NDT // 4, 128, 4 * WCOLS))
        in_maps.append({
            "hst": hst_p,
            "w": w_p,
            "bq": np.ascontiguousarray(bq_c, dtype=np.float32),
            "bkv": np.concatenate([bk[ks], bv[ks]]).reshape(128, 1)
                     .astype(np.float32),
            "ident": ident,
        })
    return in_maps


_NC_CACHE = {}


def get_nc():
    if "nc" not in _NC_CACHE:
        _NC_CACHE["nc"] = build_nc()
    return _NC_CACHE["nc"]


def kernel(hidden_states, Wq, bq, Wk, bk, Wv, bv):
    nc = get_nc()
    in_maps = make_in_maps(hidden_states, Wq, bq, Wk, bk, Wv, bv)
    res = run_bass_kernel_spmd(nc, in_maps, list(range(NCORES)))
    outs = [np.asarray(r["out"], dtype=np.float32) for r in res.results]
    return np.concatenate(outs, axis=-1)
